# revision 1
# baseline (speedup 1.0000x reference)
"""AugmentedGeneEmbedding kernel for 8 TRN2 NeuronCores (Bass/Tile).

Math (per token t with gene g = idx[t]):
    id_vec  = id_table[g]                                  # [128]
    e       = gene_idx_to_esm_idx[g]
    valid   = (g < N_GENES) & (0 < e < V_ESM)
    seq     = valid ? esm_table[e] @ Wp + bp : 0           # [256]
    h       = concat([id_vec, tanh(gate) * seq])           # [384]
    y       = gelu(h @ W1 + b1) @ W2 + b2                  # [128]

Every factor depends only on the gene, so y[t] = Y[g(t)] for a per-gene
table Y.  The kernel therefore dedups tokens to unique genes:

  Phase A (per gene, ~2500/core): fused z = id @ W1_top
      + esm_row @ (tanh(g) Wp W1_bot) + mask * (tanh(g) bp W1_bot) + b1,
      Y = gelu(z) @ W2 + b2, written fp32 to a DRAM scratch table in
      p-major row order (contiguous 2 KB per partition per tile).
  Phase B (per token): non-transpose dma_gather of 512 B fp32 Y rows by
      token, fp32->bf16 cast on DVE, contiguous per-partition writes to
      out (the host unpermutes row order during shard reassembly).

Sharding: unique genes are snake-assigned to the 8 cores balancing token
counts; each core processes the tokens of its own genes.  Tables
replicated; all gathers on device.

SWDGE queue plan: Pool-engine DMAs rotate through 8 global DMASW sems in
scheduler-emission order and each sem is locked to one queue.  We build
once with queue 0, read the emitted sem rotation, rebuild with
queue = sem % 4 per gather, and verify; fall back to single-queue if the
second schedule shifted.
"""

import numpy as np
import ml_dtypes

N_CORES = 8
B, K = 32, 2048
N_GENES, ID_DIM, ESM_DIM, PROJ, V_ESM = 20000, 128, 1280, 256, 30000
NTOK_TOTAL = B * K

NG_CAP = 2560   # unique-gene capacity per core (ceil(20000/8)=2500 padded to 5*512)
GT = 512        # genes per tile (esm gather + matmul chunk)
BIG = 1024      # tokens per phase-B gather batch (single-packet cap)

BF16 = ml_dtypes.bfloat16

_BUILD_CACHE = {}


def build_nc(n_tok, queue_plan=None):
    """Per-core Bass program (SPMD: same program on all 8 cores).
    n_tok = padded token capacity per core (multiple of 512).
    queue_plan maps gather source-index -> SWDGE queue (default all 0).
    Gather source order: esm g -> 2g, id g -> 2g+1, phase-B b -> 10+b."""
    import concourse.bacc as bacc
    import concourse.mybir as mybir
    import concourse.tile as tile
    from concourse import library_config
    from contextlib import ExitStack

    fp32 = mybir.dt.float32
    bf16 = mybir.dt.bfloat16
    i16 = mybir.dt.int16
    AF = mybir.ActivationFunctionType

    assert n_tok % 512 == 0
    n_gt = NG_CAP // GT
    qp = (queue_plan or {}).get

    nc = bacc.Bacc("TRN2", target_bir_lowering=False, num_swdge_queues=4)

    eidx_d = nc.declare_dram_parameter("eidx16", [128, NG_CAP // 16], i16, isOutput=False)
    idid_d = nc.declare_dram_parameter("idid16", [128, NG_CAP // 16], i16, isOutput=False)
    tidx_d = nc.declare_dram_parameter("tidx16", [128, n_tok // 16], i16, isOutput=False)
    mask_d = nc.declare_dram_parameter("maskbf", [1, NG_CAP], bf16, isOutput=False)
    esm_d = nc.declare_dram_parameter("esmbf", [V_ESM + 1, ESM_DIM], bf16, isOutput=False)
    id_d = nc.declare_dram_parameter("idbf", [N_GENES, ID_DIM], bf16, isOutput=False)
    wp_d = nc.declare_dram_parameter("wpbf", [ESM_DIM, PROJ], bf16, isOutput=False)
    w1_d = nc.declare_dram_parameter("w1bf", [ID_DIM + PROJ, PROJ], bf16, isOutput=False)
    w2_d = nc.declare_dram_parameter("w2bf", [PROJ, ID_DIM], bf16, isOutput=False)
    bp_d = nc.declare_dram_parameter("bpw", [128, 2], bf16, isOutput=False)
    b1_d = nc.declare_dram_parameter("b1w", [128, 2], fp32, isOutput=False)
    b2b_d = nc.declare_dram_parameter("b2b", [128, 128], fp32, isOutput=False)
    tg_d = nc.declare_dram_parameter("tgf", [128, 1], fp32, isOutput=False)
    # out row p*(n_tok/128) + off/128 + c holds token (off + c*128 + p)
    out_d = nc.declare_dram_parameter("out", [n_tok, 128], bf16, isOutput=True)

    with tile.TileContext(nc) as tc, ExitStack() as ctx:
        const = ctx.enter_context(tc.tile_pool(name="const", bufs=1))
        idp = ctx.enter_context(tc.tile_pool(name="idgat", bufs=n_gt))
        gpool = ctx.enter_context(tc.tile_pool(name="gather", bufs=n_gt))
        apool = ctx.enter_context(tc.tile_pool(name="act", bufs=4))
        ypool = ctx.enter_context(tc.tile_pool(name="yout", bufs=3))
        opool = ctx.enter_context(tc.tile_pool(name="tokout", bufs=9))
        obp = ctx.enter_context(tc.tile_pool(name="tokoutb", bufs=4))
        dram = ctx.enter_context(tc.tile_pool(name="ydram", bufs=1, space="DRAM"))
        zps = ctx.enter_context(tc.tile_pool(name="zps", bufs=4, space="PSUM"))
        yps = ctx.enter_context(tc.tile_pool(name="yps", bufs=2, space="PSUM"))

        # Gather ucode library loaded explicitly up front so the swap barrier
        # runs during the NEFF preamble instead of gating on weight DMAs.
        nc.gpsimd.load_library(library_config.mlp)

        y_dram = dram.tile([NG_CAP, 128], fp32)   # row == gene slot

        # Index tiles load on the scalar HWDGE queue so the gathers (gpsimd)
        # can start immediately; fold-critical weight loads go on sync.
        eidx_sb = const.tile([128, NG_CAP // 16], i16)
        nc.scalar.dma_start(eidx_sb[:], eidx_d[:])
        idid_sb = const.tile([128, NG_CAP // 16], i16)
        nc.scalar.dma_start(idid_sb[:], idid_d[:])
        tidx_sb = const.tile([128, n_tok // 16], i16)
        nc.scalar.dma_start(tidx_sb[:], tidx_d[:])

        # Gathers for the whole gene table issued up front; ring backpressure
        # paces them but nothing downstream needs gpsimd until phase B.
        gtiles = []
        itiles = []
        for g in range(n_gt):
            ic = g * (GT // 16)
            itile = idp.tile([128, 1, GT], bf16, tag="I", name=f"I{g}")
            nc.gpsimd.dma_gather(itile[:], id_d[:],
                                 idid_sb[:, ic:ic + GT // 16], GT, GT, ID_DIM,
                                 transpose=True, queue_num=qp(2 * g, 0))
            itiles.append(itile)
            gtile = gpool.tile([128, 10, GT], bf16, tag="G", name=f"G{g}")
            nc.gpsimd.dma_gather(gtile[:], esm_d[:],
                                 eidx_sb[:, ic:ic + GT // 16], GT, GT, ESM_DIM,
                                 transpose=True, queue_num=qp(2 * g + 1, 0))
            gtiles.append(gtile)

        # Weight loads after gather issuance in program order.
        wpT0 = const.tile([128, ESM_DIM], bf16)
        nc.sync.dma_start(wpT0[:], wp_d[:, 0:128], transpose=True)
        wpT1 = const.tile([128, ESM_DIM], bf16)
        nc.sync.dma_start(wpT1[:], wp_d[:, 128:256], transpose=True)
        w1b_sb = const.tile([128, 2, PROJ], bf16)
        nc.sync.dma_start(w1b_sb[:], w1_d[128:384, :].rearrange("(c p) f -> p c f", p=128))
        tg_sb = const.tile([128, 1], fp32)
        nc.sync.dma_start(tg_sb[:], tg_d[:])
        w1t_sb = const.tile([128, PROJ], bf16)
        nc.sync.dma_start(w1t_sb[:], w1_d[0:128, :])
        mask_sb = const.tile([1, NG_CAP], bf16)
        nc.scalar.dma_start(mask_sb[:], mask_d[:])
        w2_sb = const.tile([128, 2, 128], bf16)
        nc.scalar.dma_start(w2_sb[:], w2_d[:].rearrange("(c p) f -> p c f", p=128))
        bp_sb = const.tile([128, 2], bf16)
        nc.sync.dma_start(bp_sb[:], bp_d[:])
        b1_sb = const.tile([128, 2], fp32)
        nc.scalar.dma_start(b1_sb[:], b1_d[:])
        b2b_sb = const.tile([128, 128], fp32)
        nc.scalar.dma_start(b2b_sb[:], b2b_d[:])

        # Warm the scalar-engine activation table set containing Gelu during
        # the preamble; otherwise the table load lands mid-stream and blocks
        # the scalar FIFO (and everything downstream) until all DMAs drain.
        warm = const.tile([128, 1], fp32)
        nc.scalar.activation(warm[:], tg_sb[:], AF.Gelu)

        # ---------- one-time weight folding ----------
        wc_sb = const.tile([128, 10, PROJ], bf16)  # Wc = tanh(g) * (Wp @ W1_bot)
        cb_sb = const.tile([1, PROJ], bf16)        # cb = tanh(g) * (bp @ W1_bot)
        with tc.tile_pool(name="foldps", bufs=2, space="PSUM") as fps:
            for c in range(10):
                wc_ps = fps.tile([128, PROJ], fp32, tag="fold")
                nc.tensor.matmul(wc_ps[:], wpT0[:, c * 128:(c + 1) * 128],
                                 w1b_sb[:, 0, :], start=True, stop=False)
                nc.tensor.matmul(wc_ps[:], wpT1[:, c * 128:(c + 1) * 128],
                                 w1b_sb[:, 1, :], start=False, stop=True)
                nc.scalar.activation(wc_sb[:, c, :], wc_ps[:], AF.Copy,
                                     scale=tg_sb[:, 0:1])

            cb_ps = fps.tile([1, PROJ], fp32, tag="fold")
            nc.tensor.matmul(cb_ps[:], bp_sb[:, 0:1], w1b_sb[:, 0, :], start=True, stop=False)
            nc.tensor.matmul(cb_ps[:], bp_sb[:, 1:2], w1b_sb[:, 1, :], start=False, stop=True)
            nc.scalar.activation(cb_sb[:], cb_ps[:], AF.Copy, scale=tg_sb[0:1, 0:1])

        # ---------- phase A: per-gene table Y ----------
        for g in range(n_gt):
            gtile = gtiles[g]
            g0 = g * GT
            a_tiles = []
            for h in range(2):
                hs = slice(h * 128, (h + 1) * 128)
                zp = zps.tile([128, GT], fp32, tag="z")
                for c in range(10):
                    nc.tensor.matmul(zp[:], wc_sb[:, c, hs],
                                     gtile[:, c, :],
                                     start=c == 0, stop=False)
                # id contribution late: each tile's chain starts on esm data
                # alone, giving the (latency-bound) id gathers extra slack
                nc.tensor.matmul(zp[:], w1t_sb[:, hs], itiles[g][:, 0, :],
                                 start=False, stop=False)
                nc.tensor.matmul(zp[:], cb_sb[0:1, hs],
                                 mask_sb[0:1, g0:g0 + GT],
                                 start=False, stop=True)
                at = apool.tile([128, GT], bf16, tag="a")
                nc.scalar.activation(at[:], zp[:], AF.Gelu, bias=b1_sb[:, h:h + 1])
                a_tiles.append(at)
            ysb = ypool.tile([128, GT // 128, 128], fp32, tag="y")
            for q in range(GT // 128):
                qs = slice(q * 128, (q + 1) * 128)
                yp = yps.tile([128, 128], fp32, tag="yp")
                nc.tensor.matmul(yp[:], a_tiles[0][:, qs], w2_sb[:, 0, :],
                                 start=True, stop=False)
                nc.tensor.matmul(yp[:], a_tiles[1][:, qs], w2_sb[:, 1, :],
                                 start=False, stop=True)
                nc.vector.tensor_add(ysb[:, q, :], yp[:], b2b_sb[:])
            nc.sync.dma_start(
                y_dram[g0:g0 + GT, :].rearrange("(q p) f -> p q f", p=128), ysb[:])

        # ---------- phase B: token gather from Y ----------
        W = n_tok // 128
        outT = out_d[:].rearrange("(p w) f -> p (w f)", p=128)  # [128, n_tok]
        off = 0
        b = 0
        while off < n_tok:
            sz = min(BIG, n_tok - off)
            ot = opool.tile([128, sz // 128, 128], fp32, tag="o")
            nc.gpsimd.dma_gather(ot[:], y_dram[:, :],
                                 tidx_sb[:, off // 16:(off + sz) // 16], sz, sz, 128,
                                 elem_step=128, queue_num=qp(10 + b, 0),
                                 single_packet=sz <= 1024)
            ob = obp.tile([128, sz // 128, 128], bf16, tag="ob")
            nc.vector.tensor_copy(ob[:], ot[:])
            nc.sync.dma_start(outT[:, off:off + sz],
                              ob[:].rearrange("p a b -> p (a b)"))
            off += sz
            b += 1

    nc.compile()
    return nc


def _gather_emission(nc):
    """(source_sig, queue, sem_idx) per InstDMAGatherAnt in emission order."""
    import re
    import concourse.mybir as mybir
    out = []
    for i in nc.all_instructions():
        if type(i).__name__ != "InstDMAGatherAnt":
            continue
        sem = None
        if i.sync_info is not None:
            for u in i.sync_info.on_update:
                m = re.search(r"DMASW(\d+)_", str(u))
                if m:
                    sem = int(m.group(1))
        out.append((int(i.num_idxs), int(i.elem_size), bool(i.transpose),
                    int(i.queue_num), sem))
    return out


def _plan_queues(nc, n_tok):
    """Map gather source-index -> queue from the pass-1 sem rotation."""
    em = _gather_emission(nc)
    # expected source signatures in program order
    src = []
    for g in range(NG_CAP // GT):
        src.append((GT, ID_DIM, True))     # 2g
        src.append((GT, ESM_DIM, True))    # 2g+1
    off = 0
    while off < n_tok:
        sz = min(BIG, n_tok - off)
        src.append((sz, 128, False))       # 10+b
        off += sz
    if len(em) != len(src):
        return None
    from collections import defaultdict, deque
    pools = defaultdict(deque)
    for pos, (ni, es, tr, q, sem) in enumerate(em):
        if sem is None:
            return None
        pools[(ni, es, tr)].append(sem)
    plan = {}
    for si, sig in enumerate(src):
        if not pools[sig]:
            return None
        plan[si] = pools[sig].popleft() % 4
    return plan


def _queues_consistent(nc):
    sems = {}
    for (ni, es, tr, q, sem) in _gather_emission(nc):
        if sem is None:
            return False
        if sems.setdefault(sem, q) != q:
            return False
    return True


def _build_best(n_tok):
    nc0 = build_nc(n_tok, None)
    try:
        plan = _plan_queues(nc0, n_tok)
        if plan and any(q != 0 for q in plan.values()):
            nc1 = build_nc(n_tok, plan)
            if _queues_consistent(nc1):
                return nc1
    except Exception:
        pass
    return nc0


def _wrap16(a16):
    """int16 [n] -> [128, n//16]: logical index i at [i % 16 (+16k), i // 16]."""
    w = a16.reshape(-1, 16).T
    return np.tile(w, (8, 1)).copy()


def prepare_host(idx, gene_idx_to_esm_idx, id_table, esm_table, Wp, bp, gate,
                 W1, b1, W2, b2, n_cores=N_CORES):
    """Index prep + dtype/layout marshalling.

    Returns (shared, per_core, tok_pos, n_tok_cap); tok_pos[c] are the
    original flat token positions handled by core c, in the order the core
    emits them (sorted by Y row).
    """
    idx_flat = np.asarray(idx).reshape(-1).astype(np.int64)
    gmap = np.asarray(gene_idx_to_esm_idx).astype(np.int64)
    g_clip = np.clip(idx_flat, 0, N_GENES - 1)
    oob = (idx_flat < 0) | (idx_flat >= N_GENES)
    # key encodes (id row, forced-invalid) so OOB tokens get mask=0 entries
    key = np.where(oob, g_clip + N_GENES, g_clip)
    uniq, inv = np.unique(key, return_inverse=True)
    U = len(uniq)
    cnt = np.bincount(inv, minlength=U)

    # snake-assign genes (sorted by token count desc) to cores; slot = round
    order = np.argsort(-cnt, kind="stable")
    k = np.arange(U)
    rnd = k // n_cores
    c = k % n_cores
    core_snake = np.where(rnd % 2 == 0, c, n_cores - 1 - c)
    core_of = np.empty(U, np.int64)
    core_of[order] = core_snake
    # within each core, order genes by key value (ascending table reads)
    slot_of = np.empty(U, np.int64)
    for cc in range(n_cores):
        m = np.nonzero(core_of == cc)[0]      # ascending key order (uniq sorted)
        slot_of[m] = np.arange(len(m))
        assert len(m) <= NG_CAP

    urow = np.where(uniq >= N_GENES, uniq - N_GENES, uniq)   # id-table row
    ue = gmap[np.clip(urow, 0, N_GENES - 1)]
    uvalid = (uniq < N_GENES) & (ue > 0) & (ue < V_ESM)
    ueidx = np.where(uvalid, ue, V_ESM)                      # row V_ESM is zero pad

    eidx_core = np.full((n_cores, NG_CAP), V_ESM, np.int16)
    idid_core = np.zeros((n_cores, NG_CAP), np.int16)
    mask_core = np.zeros((n_cores, NG_CAP), BF16)
    eidx_core[core_of, slot_of] = ueidx.astype(np.int16)
    idid_core[core_of, slot_of] = urow.astype(np.int16)
    mask_core[core_of, slot_of] = uvalid.astype(BF16)

    yrow_of = slot_of                         # rank-major: Y row == slot

    tok_core = core_of[inv]
    tok_yrow = yrow_of[inv]
    tok_pos = []
    for cc in range(n_cores):
        pos = np.nonzero(tok_core == cc)[0]
        pos = pos[np.argsort(tok_yrow[pos], kind="stable")]
        tok_pos.append(pos)
    n_max = max(len(p) for p in tok_pos)
    n_tok_cap = max(512, -(-n_max // 512) * 512)

    shared = {
        "esmbf": np.concatenate(
            [np.asarray(esm_table).astype(BF16), np.zeros((1, ESM_DIM), BF16)], axis=0),
        "idbf": np.asarray(id_table).astype(BF16),
        "wpbf": np.asarray(Wp).astype(BF16),
        "w1bf": np.asarray(W1).astype(BF16),
        "w2bf": np.asarray(W2).astype(BF16),
        "bpw": np.asarray(bp).astype(BF16).reshape(2, 128).T.copy(),
        "b1w": np.asarray(b1).astype(np.float32).reshape(2, 128).T.copy(),
        "b2b": np.tile(np.asarray(b2).astype(np.float32).reshape(1, 128), (128, 1)).copy(),
        "tgf": np.full((128, 1), np.tanh(float(np.asarray(gate).reshape(-1)[0])), np.float32),
    }
    per_core = []
    for cc in range(n_cores):
        tl = np.zeros(n_tok_cap, np.int16)
        pos = tok_pos[cc]
        tl[:len(pos)] = tok_yrow[pos].astype(np.int16)
        per_core.append({
            "eidx16": _wrap16(eidx_core[cc]),
            "idid16": _wrap16(idid_core[cc]),
            "tidx16": _wrap16(tl),
            "maskbf": mask_core[cc].reshape(1, -1).copy(),
        })
    return shared, per_core, tok_pos, n_tok_cap


def _dev_rows(n, n_tok_cap):
    """DRAM out row holding sorted-token position t (first n of n_tok_cap)."""
    t = np.arange(n)
    off = (t // BIG) * BIG
    r = t - off
    return (r % 128) * (n_tok_cap // 128) + off // 128 + r // 128


def kernel(idx, gene_idx_to_esm_idx, id_table, esm_table, Wp, bp, gate,
           W1, b1, W2, b2, _trace=False, **_run_kwargs):
    from concourse.bass_utils import run_bass_kernel_spmd

    shared, per_core, tok_pos, n_tok_cap = prepare_host(
        idx, gene_idx_to_esm_idx, id_table, esm_table, Wp, bp, gate, W1, b1, W2, b2)
    if n_tok_cap not in _BUILD_CACHE:
        _BUILD_CACHE[n_tok_cap] = _build_best(n_tok_cap)
    nc = _BUILD_CACHE[n_tok_cap]

    in_maps = [dict(shared, **pc) for pc in per_core]
    res = run_bass_kernel_spmd(nc, in_maps, list(range(N_CORES)), trace=_trace,
                               **_run_kwargs)
    sh = np.asarray(idx).shape
    out = np.empty((NTOK_TOTAL, ID_DIM), np.float32)
    for c in range(N_CORES):
        pos = tok_pos[c]
        rows = np.asarray(res.results[c]["out"])
        out[pos] = rows[_dev_rows(len(pos), n_tok_cap)].astype(np.float32)
    out = out.reshape(sh[0], sh[1], ID_DIM)
    if _trace:
        return out, res
    return out



# revision 2
# speedup vs baseline: 1.8152x; 1.8152x over previous
"""AugmentedGeneEmbedding kernel for 8 TRN2 NeuronCores (Bass/Tile).

Math (per token t with gene g = idx[t]):
    id_vec  = id_table[g]                                  # [128]
    e       = gene_idx_to_esm_idx[g]
    valid   = (g < N_GENES) & (0 < e < V_ESM)
    seq     = valid ? esm_table[e] @ Wp + bp : 0           # [256]
    h       = concat([id_vec, tanh(gate) * seq])           # [384]
    y       = gelu(h @ W1 + b1) @ W2 + b2                  # [128]

Every factor depends only on the gene, so y[t] = Y[g(t)] for a per-gene
table Y.  The kernel dedups tokens to unique genes and computes Y once
per gene, then expands Y to tokens with one-hot selection matmuls (no
token gather at all):

  Host: fold Wc = tanh(g) * (Wp @ W1_bot)  (and cb = tanh(g) * bp @
      W1_bot, zero for this input); snake-assign unique genes to
      8 cores x 20 slot-groups of 128 slots each, balancing token
      counts (each group ends up with <=128 genes and <=512 tokens);
      build per-group one-hot SEL[slot, tok] matrices in bf16.
  Phase A (device, per 512-gene tile): gather esm+id rows (transposing
      SWDGE gather), z = Wc.T@esm + W1_top.T@id (+ mask*cb) ,
      a = gelu(z + b1), Y_q = a @ W2 + b2 -> SBUF bf16 [slot, feat].
  Phase B (device, per 128-slot group, fused into phase A): one PE
      matmul out[feat, tok] = Y_q.T @ SEL_q, DVE copy to bf16, write
      out columns.  Runs immediately after each Y chunk; no DRAM
      round-trip, no phase-B gathers.

SWDGE queue plan: the 10 gathers rotate through global DMASW sems in
scheduler-emission order; we build once with queue 0, read the emitted
sem rotation, rebuild with queue = sem % 4, verify, else fall back.
"""

import numpy as np
import ml_dtypes

N_CORES = 8
B, K = 32, 2048
N_GENES, ID_DIM, ESM_DIM, PROJ, V_ESM = 20000, 128, 1280, 256, 30000
NTOK_TOTAL = B * K

NG_CAP = 2560   # unique-gene slot capacity per core (20 groups of 128)
GT = 512        # genes per tile (esm gather + matmul chunk); 4 groups
N_GROUPS = NG_CAP // 128          # 20 slot-groups per core
TPG = 512                         # token columns per group (padded)

BF16 = ml_dtypes.bfloat16

_BUILD_CACHE = {}


def build_nc(has_cb, queue_plan=None):
    """Per-core Bass program (SPMD: same program on all 8 cores).
    queue_plan maps gather source-index -> SWDGE queue (default all 0).
    Gather source order: esm g -> 2g, id g -> 2g+1."""
    import concourse.bacc as bacc
    import concourse.mybir as mybir
    import concourse.tile as tile
    from concourse import library_config
    from contextlib import ExitStack

    fp32 = mybir.dt.float32
    bf16 = mybir.dt.bfloat16
    i16 = mybir.dt.int16
    AF = mybir.ActivationFunctionType

    n_gt = NG_CAP // GT
    qp = (queue_plan or {}).get

    nc = bacc.Bacc("TRN2", target_bir_lowering=False, num_swdge_queues=4)

    eidx_d = nc.declare_dram_parameter("eidx16", [128, NG_CAP // 16], i16, isOutput=False)
    idid_d = nc.declare_dram_parameter("idid16", [128, NG_CAP // 16], i16, isOutput=False)
    sel_d = nc.declare_dram_parameter("selbf", [NG_CAP, TPG], bf16, isOutput=False)
    esm_d = nc.declare_dram_parameter("esmbf", [V_ESM + 1, ESM_DIM], bf16, isOutput=False)
    id_d = nc.declare_dram_parameter("idbf", [N_GENES, ID_DIM], bf16, isOutput=False)
    wc_d = nc.declare_dram_parameter("wcbf", [128, 10 * PROJ], bf16, isOutput=False)
    w1t_d = nc.declare_dram_parameter("w1tbf", [128, PROJ], bf16, isOutput=False)
    w2_d = nc.declare_dram_parameter("w2bf", [128, 2 * ID_DIM], bf16, isOutput=False)
    b1_d = nc.declare_dram_parameter("b1w", [128, 2], fp32, isOutput=False)
    b2b_d = nc.declare_dram_parameter("b2b", [128, 128], fp32, isOutput=False)
    if has_cb:
        mask_d = nc.declare_dram_parameter("maskbf", [1, NG_CAP], bf16, isOutput=False)
        cb_d = nc.declare_dram_parameter("cbbf", [1, PROJ], bf16, isOutput=False)
    # out column q*TPG + j holds token j of slot-group q (features on rows)
    out_d = nc.declare_dram_parameter("out", [128, N_GROUPS * TPG], bf16, isOutput=True)

    with tile.TileContext(nc) as tc, ExitStack() as ctx:
        const = ctx.enter_context(tc.tile_pool(name="const", bufs=1))
        idp = ctx.enter_context(tc.tile_pool(name="idgat", bufs=n_gt))
        gpool = ctx.enter_context(tc.tile_pool(name="gather", bufs=n_gt))
        apool = ctx.enter_context(tc.tile_pool(name="act", bufs=4))
        ypool = ctx.enter_context(tc.tile_pool(name="ygrp", bufs=3))
        opool = ctx.enter_context(tc.tile_pool(name="tokout", bufs=3))
        zps = ctx.enter_context(tc.tile_pool(name="zps", bufs=3, space="PSUM"))
        yps = ctx.enter_context(tc.tile_pool(name="yps", bufs=2, space="PSUM"))
        bps = ctx.enter_context(tc.tile_pool(name="bps", bufs=3, space="PSUM"))

        # Gather ucode library loaded explicitly up front so the swap barrier
        # runs during the NEFF preamble instead of gating on weight DMAs.
        nc.gpsimd.load_library(library_config.mlp)

        # Index tiles load first on the sync HWDGE ring so the gathers
        # (gpsimd) can start immediately; everything else queues behind.
        eidx_sb = const.tile([128, NG_CAP // 16], i16)
        nc.sync.dma_start(eidx_sb[:], eidx_d[:])
        idid_sb = const.tile([128, NG_CAP // 16], i16)
        nc.sync.dma_start(idid_sb[:], idid_d[:])

        # Warm the scalar-engine activation table set containing Gelu during
        # the preamble; otherwise the table load lands mid-stream and blocks
        # the scalar FIFO (and everything downstream) until all DMAs drain.
        b1_sb = const.tile([128, 2], fp32)
        nc.scalar.dma_start(b1_sb[:], b1_d[:])
        warm = const.tile([128, 1], fp32)
        nc.scalar.activation(warm[:], b1_sb[:, 0:1], AF.Gelu)

        # Gathers for the whole gene table issued up front; ring backpressure
        # paces them.  esm before id per tile: the z chain consumes esm
        # chunks first, id only at the end.
        gtiles = []
        itiles = []
        for g in range(n_gt):
            ic = g * (GT // 16)
            gtile = gpool.tile([128, 10, GT], bf16, tag="G", name=f"G{g}")
            nc.gpsimd.dma_gather(gtile[:], esm_d[:],
                                 eidx_sb[:, ic:ic + GT // 16], GT, GT, ESM_DIM,
                                 transpose=True, queue_num=qp(2 * g, 0))
            gtiles.append(gtile)
            itile = idp.tile([128, 1, GT], bf16, tag="I", name=f"I{g}")
            nc.gpsimd.dma_gather(itile[:], id_d[:],
                                 idid_sb[:, ic:ic + GT // 16], GT, GT, ID_DIM,
                                 transpose=True, queue_num=qp(2 * g + 1, 0))
            itiles.append(itile)

        # Weight loads after gather issuance in program order.
        wc_sb = const.tile([128, 10, PROJ], bf16)   # Wc = tanh(g)*(Wp@W1_bot)
        nc.sync.dma_start(wc_sb[:].rearrange("p c f -> p (c f)"), wc_d[:])
        w1t_sb = const.tile([128, PROJ], bf16)
        nc.sync.dma_start(w1t_sb[:], w1t_d[:])
        w2_sb = const.tile([128, 2, ID_DIM], bf16)
        nc.scalar.dma_start(w2_sb[:].rearrange("p c f -> p (c f)"), w2_d[:])
        b2b_sb = const.tile([128, 128], fp32)
        nc.scalar.dma_start(b2b_sb[:], b2b_d[:])
        if has_cb:
            mask_sb = const.tile([1, NG_CAP], bf16)
            nc.scalar.dma_start(mask_sb[:], mask_d[:])
            cb_sb = const.tile([1, PROJ], bf16)
            nc.scalar.dma_start(cb_sb[:], cb_d[:])
        sel_sb = []
        for q in range(N_GROUPS):
            st = const.tile([128, TPG], bf16, name=f"SEL{q}")
            nc.scalar.dma_start(st[:], sel_d[q * 128:(q + 1) * 128, :])
            sel_sb.append(st)

        # ---------- fused phase A (per-gene Y) + phase B (token expand) ----
        for g in range(n_gt):
            gtile = gtiles[g]
            g0 = g * GT
            a_tiles = []
            for h in range(2):
                hs = slice(h * 128, (h + 1) * 128)
                zp = zps.tile([128, GT], fp32, tag="z")
                for c in range(10):
                    nc.tensor.matmul(zp[:], wc_sb[:, c, hs], gtile[:, c, :],
                                     start=c == 0, stop=False)
                # id contribution late: each chain starts on esm data alone,
                # giving the (latency-bound) id gathers extra slack
                nc.tensor.matmul(zp[:], w1t_sb[:, hs], itiles[g][:, 0, :],
                                 start=False, stop=not has_cb)
                if has_cb:
                    nc.tensor.matmul(zp[:], cb_sb[0:1, hs],
                                     mask_sb[0:1, g0:g0 + GT],
                                     start=False, stop=True)
                at = apool.tile([128, GT], bf16, tag="a")
                nc.scalar.activation(at[:], zp[:], AF.Gelu, bias=b1_sb[:, h:h + 1])
                a_tiles.append(at)
            osb = opool.tile([128, GT // 128, TPG], bf16, tag="o")
            for qq in range(GT // 128):
                qs = slice(qq * 128, (qq + 1) * 128)
                yp = yps.tile([128, 128], fp32, tag="yp")
                nc.tensor.matmul(yp[:], a_tiles[0][:, qs], w2_sb[:, 0, :],
                                 start=True, stop=False)
                nc.tensor.matmul(yp[:], a_tiles[1][:, qs], w2_sb[:, 1, :],
                                 start=False, stop=True)
                yq = ypool.tile([128, 128], bf16, tag="y")
                nc.vector.tensor_add(yq[:], yp[:], b2b_sb[:])
                # phase B for this slot-group: one-hot selection matmul
                q = g * (GT // 128) + qq
                bb = bps.tile([128, TPG], fp32, tag="b")
                nc.tensor.matmul(bb[:], yq[:], sel_sb[q][:], start=True, stop=True)
                nc.vector.tensor_copy(osb[:, qq, :], bb[:])
            nc.sync.dma_start(out_d[:, g * (GT // 128) * TPG:(g + 1) * (GT // 128) * TPG],
                              osb[:].rearrange("p a b -> p (a b)"))

    nc.compile()
    return nc


def _gather_emission(nc):
    """(num_idxs, elem_size, transpose, queue, sem_idx) per InstDMAGatherAnt
    in emission order."""
    import re
    out = []
    for i in nc.all_instructions():
        if type(i).__name__ != "InstDMAGatherAnt":
            continue
        sem = None
        if i.sync_info is not None:
            for u in i.sync_info.on_update:
                m = re.search(r"DMASW(\d+)_", str(u))
                if m:
                    sem = int(m.group(1))
        out.append((int(i.num_idxs), int(i.elem_size), bool(i.transpose),
                    int(i.queue_num), sem))
    return out


def _plan_queues(nc):
    """Map gather source-index -> queue from the pass-1 sem rotation."""
    em = _gather_emission(nc)
    src = []
    for g in range(NG_CAP // GT):
        src.append((GT, ESM_DIM, True))    # 2g
        src.append((GT, ID_DIM, True))     # 2g+1
    if len(em) != len(src):
        return None
    from collections import defaultdict, deque
    pools = defaultdict(deque)
    for (ni, es, tr, q, sem) in em:
        if sem is None:
            return None
        pools[(ni, es, tr)].append(sem)
    plan = {}
    for si, sig in enumerate(src):
        if not pools[sig]:
            return None
        plan[si] = pools[sig].popleft() % 4
    return plan


def _queues_consistent(nc):
    sems = {}
    for (ni, es, tr, q, sem) in _gather_emission(nc):
        if sem is None:
            return False
        if sems.setdefault(sem, q) != q:
            return False
    return True


def _build_best(has_cb):
    nc0 = build_nc(has_cb, None)
    try:
        plan = _plan_queues(nc0)
        if plan and any(q != 0 for q in plan.values()):
            nc1 = build_nc(has_cb, plan)
            if _queues_consistent(nc1):
                return nc1
    except Exception:
        pass
    return nc0


def _wrap16(a16):
    """int16 [n] -> [128, n//16]: logical index i at [i % 16 (+16k), i // 16]."""
    w = a16.reshape(-1, 16).T
    return np.tile(w, (8, 1)).copy()


def prepare_host(idx, gene_idx_to_esm_idx, id_table, esm_table, Wp, bp, gate,
                 W1, b1, W2, b2, n_cores=N_CORES):
    """Index prep, weight folding, dtype/layout marshalling.

    Returns (shared, per_core, tok_pos, has_cb); tok_pos[c][q] are the
    original flat token positions in slot-group q of core c, in SEL column
    order."""
    idx_flat = np.asarray(idx).reshape(-1).astype(np.int64)
    gmap = np.asarray(gene_idx_to_esm_idx).astype(np.int64)
    g_clip = np.clip(idx_flat, 0, N_GENES - 1)
    oob = (idx_flat < 0) | (idx_flat >= N_GENES)
    # key encodes (id row, forced-invalid) so OOB tokens get mask=0 entries
    key = np.where(oob, g_clip + N_GENES, g_clip)
    uniq, inv = np.unique(key, return_inverse=True)
    U = len(uniq)
    cnt = np.bincount(inv, minlength=U)

    # snake-assign genes (sorted by token count desc) to the 160 (core,
    # group) bins; each bin <=128 genes and (with this input) <=~420 tokens
    NB = n_cores * N_GROUPS
    order = np.argsort(-cnt, kind="stable")
    k = np.arange(U)
    rnd = k // NB
    c = k % NB
    bin_snake = np.where(rnd % 2 == 0, c, NB - 1 - c)
    bin_of = np.empty(U, np.int64)
    bin_of[order] = bin_snake
    core_of = bin_of % n_cores
    grp_of = bin_of // n_cores
    # within each bin, order genes by key value (ascending table reads)
    rank_of = np.empty(U, np.int64)
    for b in range(NB):
        m = np.nonzero(bin_of == b)[0]        # ascending key order
        assert len(m) <= 128, f"bin {b} has {len(m)} genes"
        rank_of[m] = np.arange(len(m))
    slot_of = grp_of * 128 + rank_of

    urow = np.where(uniq >= N_GENES, uniq - N_GENES, uniq)   # id-table row
    ue = gmap[np.clip(urow, 0, N_GENES - 1)]
    uvalid = (uniq < N_GENES) & (ue > 0) & (ue < V_ESM)
    ueidx = np.where(uvalid, ue, V_ESM)                      # row V_ESM is zero pad

    eidx_core = np.full((n_cores, NG_CAP), V_ESM, np.int16)
    idid_core = np.zeros((n_cores, NG_CAP), np.int16)
    mask_core = np.zeros((n_cores, NG_CAP), BF16)
    eidx_core[core_of, slot_of] = ueidx.astype(np.int16)
    idid_core[core_of, slot_of] = urow.astype(np.int16)
    mask_core[core_of, slot_of] = uvalid.astype(BF16)

    # tokens -> SEL one-hots: column j of (core, group) = j-th token of that
    # bin in flat order
    tok_bin = bin_of[inv]
    tok_rank = rank_of[inv]
    bin_sort = np.argsort(tok_bin, kind="stable")  # flat positions by bin
    bcnt = np.bincount(tok_bin, minlength=NB)
    boff = np.concatenate([[0], np.cumsum(bcnt)])
    sel_core = np.zeros((n_cores, NG_CAP, TPG), BF16)
    tok_pos = [[None] * N_GROUPS for _ in range(n_cores)]
    for b in range(NB):
        assert bcnt[b] <= TPG, f"bin {b} has {bcnt[b]} tokens"
        pos = bin_sort[boff[b]:boff[b + 1]]
        cc, q = b % n_cores, b // n_cores
        tok_pos[cc][q] = pos
        sel_core[cc, q * 128 + tok_rank[pos], np.arange(len(pos))] = 1

    # host weight folding
    tg = np.tanh(float(np.asarray(gate).reshape(-1)[0]))
    Wp64 = np.asarray(Wp, np.float64)
    W1b = np.asarray(W1, np.float64)[ID_DIM:, :]
    Wc = tg * (Wp64 @ W1b)                                   # [1280, 256]
    cb = tg * (np.asarray(bp, np.float64) @ W1b)             # [256]
    has_cb = bool(np.abs(cb).max() > 1e-12)

    shared = {
        "esmbf": np.concatenate(
            [np.asarray(esm_table).astype(BF16), np.zeros((1, ESM_DIM), BF16)], axis=0),
        "idbf": np.asarray(id_table).astype(BF16),
        "wcbf": Wc.reshape(10, 128, PROJ).transpose(1, 0, 2).reshape(128, 10 * PROJ)
                  .astype(BF16).copy(),
        "w1tbf": np.asarray(W1[:ID_DIM, :]).astype(BF16),
        "w2bf": np.asarray(W2).reshape(2, 128, ID_DIM).transpose(1, 0, 2)
                  .reshape(128, 2 * ID_DIM).astype(BF16).copy(),
        "b1w": np.asarray(b1).astype(np.float32).reshape(2, 128).T.copy(),
        "b2b": np.tile(np.asarray(b2).astype(np.float32).reshape(1, 128), (128, 1)).copy(),
    }
    if has_cb:
        shared["cbbf"] = cb.astype(BF16).reshape(1, PROJ).copy()
    per_core = []
    for cc in range(n_cores):
        pc = {
            "eidx16": _wrap16(eidx_core[cc]),
            "idid16": _wrap16(idid_core[cc]),
            "selbf": sel_core[cc],
        }
        if has_cb:
            pc["maskbf"] = mask_core[cc].reshape(1, -1).copy()
        per_core.append(pc)
    return shared, per_core, tok_pos, has_cb


def kernel(idx, gene_idx_to_esm_idx, id_table, esm_table, Wp, bp, gate,
           W1, b1, W2, b2, _trace=False, **_run_kwargs):
    from concourse.bass_utils import run_bass_kernel_spmd

    shared, per_core, tok_pos, has_cb = prepare_host(
        idx, gene_idx_to_esm_idx, id_table, esm_table, Wp, bp, gate, W1, b1, W2, b2)
    if has_cb not in _BUILD_CACHE:
        _BUILD_CACHE[has_cb] = _build_best(has_cb)
    nc = _BUILD_CACHE[has_cb]

    in_maps = [dict(shared, **pc) for pc in per_core]
    res = run_bass_kernel_spmd(nc, in_maps, list(range(N_CORES)), trace=_trace,
                               **_run_kwargs)
    sh = np.asarray(idx).shape
    out = np.empty((NTOK_TOTAL, ID_DIM), np.float32)
    for c in range(N_CORES):
        arr = np.asarray(res.results[c]["out"]).astype(np.float32)  # [128, 20*TPG]
        for q in range(N_GROUPS):
            pos = tok_pos[c][q]
            if len(pos):
                out[pos] = arr[:, q * TPG:q * TPG + len(pos)].T
    out = out.reshape(sh[0], sh[1], ID_DIM)
    if _trace:
        return out, res
    return out


# revision 4
# speedup vs baseline: 1.8246x; 1.0051x over previous
"""AugmentedGeneEmbedding kernel for 8 TRN2 NeuronCores (Bass/Tile).

Math (per token t with gene g = idx[t]):
    id_vec  = id_table[g]                                  # [128]
    e       = gene_idx_to_esm_idx[g]
    valid   = (g < N_GENES) & (0 < e < V_ESM)
    seq     = valid ? esm_table[e] @ Wp + bp : 0           # [256]
    h       = concat([id_vec, tanh(gate) * seq])           # [384]
    y       = gelu(h @ W1 + b1) @ W2 + b2                  # [128]

Every factor depends only on the gene, so y[t] = Y[g(t)] for a per-gene
table Y.  The kernel dedups tokens to unique genes and computes Y once
per gene, then expands Y to tokens with one-hot selection matmuls (no
token gather at all):

  Host: fold Wc = tanh(g) * (Wp @ W1_bot)  (and cb = tanh(g) * bp @
      W1_bot, zero for this input); snake-assign unique genes to
      8 cores x G slot-groups of 128 slots each, balancing token counts
      (each group ends up with <=128 genes and <=512 tokens); build
      per-group one-hot SEL[slot, tok] matrices in bf16.  G is the
      smallest group count that fits the unique genes (19 here).
  Phase A (device, per gene tile of <=4 groups): gather esm+id rows
      (transposing SWDGE gather), z = Wc.T@esm + W1_top.T@id
      (+ mask*cb), a = gelu(z + b1), Y_q = a @ W2 + b2 -> SBUF bf16.
  Phase B (device, per 128-slot group, fused into phase A): one PE
      matmul out[feat, tok] = Y_q.T @ SEL_q, DVE copy to bf16, write
      out columns.  Runs immediately after each Y chunk; no DRAM
      round-trip, no phase-B gathers.

Startup is dominated by the fixed NEFF preamble (~6us) and the gpsimd
gather-ucode IRAM load (~6us), so all constant loads are coalesced into
4 HWDGE DMAs (idx / hot weights / f32 biases / SEL) to keep the rings
clear while the ucode loads, and the per-gather index-count register is
hoisted (one MOVE instead of ten).

SWDGE queue plan: gathers rotate through global DMASW sems in
scheduler-emission order; we build once with queue 0, read the emitted
sem rotation, rebuild with queue = sem % 4, verify, else fall back.
"""

import numpy as np
import ml_dtypes

N_CORES = 8
B, K = 32, 2048
N_GENES, ID_DIM, ESM_DIM, PROJ, V_ESM = 20000, 128, 1280, 256, 30000
NTOK_TOTAL = B * K

TPG = 512                         # token columns per slot-group (padded)

BF16 = ml_dtypes.bfloat16

_BUILD_CACHE = {}


def _tile_groups(n_groups):
    """Split n_groups slot-groups into gene tiles of <=4 groups."""
    out = []
    g = 0
    while g < n_groups:
        out.append(min(4, n_groups - g))
        g += 4
    return out


def build_nc(n_groups, has_cb, queue_plan=None):
    """Per-core Bass program (SPMD: same program on all 8 cores).
    queue_plan maps gather source-index -> SWDGE queue (default all 0).
    Gather source order: esm tile t -> 2t, id tile t -> 2t+1."""
    import concourse.bacc as bacc
    import concourse.mybir as mybir
    import concourse.tile as tile
    from concourse import library_config
    from contextlib import ExitStack

    fp32 = mybir.dt.float32
    bf16 = mybir.dt.bfloat16
    i16 = mybir.dt.int16
    AF = mybir.ActivationFunctionType

    ng_cap = n_groups * 128
    tiles = _tile_groups(n_groups)
    n_gt = len(tiles)
    qp = (queue_plan or {}).get
    W16 = ng_cap // 16
    HOT = 10 * PROJ + PROJ + 2 * ID_DIM    # wc | w1t | w2  (bf16 cols)

    nc = bacc.Bacc("TRN2", target_bir_lowering=False, num_swdge_queues=4)

    idx_d = nc.declare_dram_parameter("idx16", [128, 2 * W16], i16, isOutput=False)
    hot_d = nc.declare_dram_parameter("hotbf", [128, HOT], bf16, isOutput=False)
    f32_d = nc.declare_dram_parameter("f32w", [128, 130], fp32, isOutput=False)
    sel_d = nc.declare_dram_parameter("selbf", [128, n_groups * TPG], bf16, isOutput=False)
    esm_d = nc.declare_dram_parameter("esmbf", [V_ESM + 1, ESM_DIM], bf16, isOutput=False)
    id_d = nc.declare_dram_parameter("idbf", [N_GENES, ID_DIM], bf16, isOutput=False)
    if has_cb:
        mcb_d = nc.declare_dram_parameter("mcbbf", [1, ng_cap + PROJ], bf16, isOutput=False)
    # out column q*TPG + j holds token j of slot-group q (features on rows)
    out_d = nc.declare_dram_parameter("out", [128, n_groups * TPG], bf16, isOutput=True)

    with tile.TileContext(nc) as tc, ExitStack() as ctx:
        const = ctx.enter_context(tc.tile_pool(name="const", bufs=1))
        idp = ctx.enter_context(tc.tile_pool(name="idgat", bufs=n_gt))
        gpool = ctx.enter_context(tc.tile_pool(name="gather", bufs=n_gt))
        apool = ctx.enter_context(tc.tile_pool(name="act", bufs=4))
        ypool = ctx.enter_context(tc.tile_pool(name="ygrp", bufs=3))
        opool = ctx.enter_context(tc.tile_pool(name="tokout", bufs=3))
        zps = ctx.enter_context(tc.tile_pool(name="zps", bufs=3, space="PSUM"))
        yps = ctx.enter_context(tc.tile_pool(name="yps", bufs=2, space="PSUM"))
        bps = ctx.enter_context(tc.tile_pool(name="bps", bufs=3, space="PSUM"))

        # Gather ucode library load first: its ~6us IRAM DMA is the gather
        # critical path, so only the (tiny) index load competes with it.
        nc.gpsimd.load_library(library_config.mlp)

        idx_sb = const.tile([128, 2 * W16], i16)
        nc.sync.dma_start(idx_sb[:], idx_d[:])
        eidx_sb = idx_sb[:, 0:W16]
        idid_sb = idx_sb[:, W16:2 * W16]

        # Warm the scalar-engine activation table set containing Gelu during
        # the preamble; otherwise the table load lands mid-stream and blocks
        # the scalar FIFO (and everything downstream) until all DMAs drain.
        f32_sb = const.tile([128, 130], fp32)
        nc.scalar.dma_start(f32_sb[:], f32_d[:])
        b1_sb = f32_sb[:, 0:2]
        b2b_sb = f32_sb[:, 2:130]
        warm = const.tile([128, 1], fp32)
        nc.scalar.activation(warm[:], b1_sb[:, 0:1], AF.Gelu, bias=b1_sb[:, 0:1])

        # Gathers for the whole gene table issued up front; ring backpressure
        # paces them.  esm before id per tile: the z chain consumes esm
        # chunks first, id only at the end.
        nreg = {}
        for gt in sorted(set(tiles)):
            nreg[gt] = nc.gpsimd.compute_val(gt * 128)
        gtiles = []
        itiles = []
        goff = 0
        for t, ngrp in enumerate(tiles):
            gn = ngrp * 128
            ic = goff * 8                  # 128 slots = 8 idx cols
            gtile = gpool.tile([128, 10, gn], bf16, tag="G", name=f"G{t}")
            nc.gpsimd.dma_gather(gtile[:], esm_d[:],
                                 eidx_sb[:, ic:ic + gn // 16], gn, nreg[ngrp],
                                 ESM_DIM, transpose=True, queue_num=qp(2 * t, 0))
            gtiles.append(gtile)
            itile = idp.tile([128, 1, gn], bf16, tag="I", name=f"I{t}")
            nc.gpsimd.dma_gather(itile[:], id_d[:],
                                 idid_sb[:, ic:ic + gn // 16], gn, nreg[ngrp],
                                 ID_DIM, transpose=True, queue_num=qp(2 * t + 1, 0))
            itiles.append(itile)
            goff += ngrp

        # Weight loads after gather issuance in program order.
        hot_sb = const.tile([128, HOT], bf16)
        nc.sync.dma_start(hot_sb[:], hot_d[:])
        wc_sb = hot_sb[:, 0:10 * PROJ]               # [(c, f)] flat
        w1t_sb = hot_sb[:, 10 * PROJ:10 * PROJ + PROJ]
        w2_sb = hot_sb[:, 11 * PROJ:11 * PROJ + 2 * ID_DIM]
        if has_cb:
            mcb_sb = const.tile([1, ng_cap + PROJ], bf16)
            nc.scalar.dma_start(mcb_sb[:], mcb_d[:])
            mask_sb = mcb_sb[:, 0:ng_cap]
            cb_sb = mcb_sb[:, ng_cap:]
        sel_sb = const.tile([128, n_groups * TPG], bf16)
        nc.scalar.dma_start(sel_sb[:], sel_d[:])

        # ---------- fused phase A (per-gene Y) + phase B (token expand) ----
        goff = 0
        for t, ngrp in enumerate(tiles):
            gn = ngrp * 128
            gtile = gtiles[t]
            a_tiles = []
            for h in range(2):
                hs = slice(h * 128, (h + 1) * 128)
                zp = zps.tile([128, gn], fp32, tag="z", name=f"z{t}_{h}")
                for c in range(10):
                    nc.tensor.matmul(zp[:], wc_sb[:, c * PROJ + h * 128:
                                                  c * PROJ + h * 128 + 128],
                                     gtile[:, c, :], start=c == 0, stop=False)
                # id contribution late: each chain starts on esm data alone,
                # giving the (latency-bound) id gathers extra slack
                nc.tensor.matmul(zp[:], w1t_sb[:, hs], itiles[t][:, 0, :],
                                 start=False, stop=not has_cb)
                if has_cb:
                    nc.tensor.matmul(zp[:], cb_sb[0:1, hs],
                                     mask_sb[0:1, goff * 128:goff * 128 + gn],
                                     start=False, stop=True)
                at = apool.tile([128, gn], bf16, tag="a", name=f"a{t}_{h}")
                nc.scalar.activation(at[:], zp[:], AF.Gelu, bias=b1_sb[:, h:h + 1])
                a_tiles.append(at)
            osb = opool.tile([128, ngrp, TPG], bf16, tag="o", name=f"o{t}")
            for qq in range(ngrp):
                qs = slice(qq * 128, (qq + 1) * 128)
                yp = yps.tile([128, 128], fp32, tag="yp")
                nc.tensor.matmul(yp[:], a_tiles[0][:, qs], w2_sb[:, 0:ID_DIM],
                                 start=True, stop=False)
                nc.tensor.matmul(yp[:], a_tiles[1][:, qs], w2_sb[:, ID_DIM:],
                                 start=False, stop=True)
                yq = ypool.tile([128, 128], bf16, tag="y")
                nc.vector.tensor_add(yq[:], yp[:], b2b_sb[:])
                # phase B for this slot-group: one-hot selection matmul
                q = goff + qq
                bb = bps.tile([128, TPG], fp32, tag="b")
                nc.tensor.matmul(bb[:], yq[:], sel_sb[:, q * TPG:(q + 1) * TPG],
                                 start=True, stop=True)
                nc.vector.tensor_copy(osb[:, qq, :], bb[:])
            nc.sync.dma_start(out_d[:, goff * TPG:(goff + ngrp) * TPG],
                              osb[:].rearrange("p a b -> p (a b)"))
            goff += ngrp

    nc.compile()
    return nc


def _gather_emission(nc):
    """(num_idxs, elem_size, transpose, queue, sem_idx) per InstDMAGatherAnt
    in emission order."""
    import re
    out = []
    for i in nc.all_instructions():
        if type(i).__name__ != "InstDMAGatherAnt":
            continue
        sem = None
        if i.sync_info is not None:
            for u in i.sync_info.on_update:
                m = re.search(r"DMASW(\d+)_", str(u))
                if m:
                    sem = int(m.group(1))
        out.append((int(i.num_idxs), int(i.elem_size), bool(i.transpose),
                    int(i.queue_num), sem))
    return out


def _plan_queues(nc, n_groups):
    """Map gather source-index -> queue from the pass-1 sem rotation."""
    em = _gather_emission(nc)
    src = []
    for ngrp in _tile_groups(n_groups):
        src.append((ngrp * 128, ESM_DIM, True))
        src.append((ngrp * 128, ID_DIM, True))
    if len(em) != len(src):
        return None
    from collections import defaultdict, deque
    pools = defaultdict(deque)
    for (ni, es, tr, q, sem) in em:
        if sem is None:
            return None
        pools[(ni, es, tr)].append(sem)
    plan = {}
    for si, sig in enumerate(src):
        if not pools[sig]:
            return None
        plan[si] = pools[sig].popleft() % 4
    return plan


def _queues_consistent(nc):
    sems = {}
    for (ni, es, tr, q, sem) in _gather_emission(nc):
        if sem is None:
            return False
        if sems.setdefault(sem, q) != q:
            return False
    return True


def _build_best(n_groups, has_cb):
    nc0 = build_nc(n_groups, has_cb, None)
    try:
        plan = _plan_queues(nc0, n_groups)
        if plan and any(q != 0 for q in plan.values()):
            nc1 = build_nc(n_groups, has_cb, plan)
            if _queues_consistent(nc1):
                return nc1
    except Exception:
        pass
    return nc0


def _wrap16(a16):
    """int16 [n] -> [128, n//16]: logical index i at [i % 16 (+16k), i // 16]."""
    w = a16.reshape(-1, 16).T
    return np.tile(w, (8, 1)).copy()


def _assign_bins(cnt, n_cores, n_groups):
    """Snake-assign genes (by count desc) to n_cores*n_groups bins.
    Returns (bin_of, ok): ok=False if any bin exceeds 128 genes or TPG
    tokens."""
    U = len(cnt)
    NB = n_cores * n_groups
    order = np.argsort(-cnt, kind="stable")
    k = np.arange(U)
    rnd = k // NB
    c = k % NB
    bin_snake = np.where(rnd % 2 == 0, c, NB - 1 - c)
    bin_of = np.empty(U, np.int64)
    bin_of[order] = bin_snake
    gcnt = np.bincount(bin_of, minlength=NB)
    tcnt = np.bincount(bin_of, weights=cnt, minlength=NB)
    return bin_of, bool(gcnt.max() <= 128 and tcnt.max() <= TPG)


def prepare_host(idx, gene_idx_to_esm_idx, id_table, esm_table, Wp, bp, gate,
                 W1, b1, W2, b2, n_cores=N_CORES):
    """Index prep, weight folding, dtype/layout marshalling.

    Returns (shared, per_core, tok_pos, n_groups, has_cb); tok_pos[c][q]
    are the original flat token positions in slot-group q of core c, in
    SEL column order."""
    idx_flat = np.asarray(idx).reshape(-1).astype(np.int64)
    gmap = np.asarray(gene_idx_to_esm_idx).astype(np.int64)
    g_clip = np.clip(idx_flat, 0, N_GENES - 1)
    oob = (idx_flat < 0) | (idx_flat >= N_GENES)
    # key encodes (id row, forced-invalid) so OOB tokens get mask=0 entries
    key = np.where(oob, g_clip + N_GENES, g_clip)
    uniq, inv = np.unique(key, return_inverse=True)
    U = len(uniq)
    cnt = np.bincount(inv, minlength=U)

    n_groups = -(-U // (128 * n_cores))
    bin_of, ok = _assign_bins(cnt, n_cores, n_groups)
    while not ok:
        n_groups += 1
        bin_of, ok = _assign_bins(cnt, n_cores, n_groups)
    NB = n_cores * n_groups
    ng_cap = n_groups * 128
    core_of = bin_of % n_cores
    grp_of = bin_of // n_cores
    # within each bin, order genes by key value (ascending table reads)
    rank_of = np.empty(U, np.int64)
    for b in range(NB):
        m = np.nonzero(bin_of == b)[0]        # ascending key order
        rank_of[m] = np.arange(len(m))
    slot_of = grp_of * 128 + rank_of

    urow = np.where(uniq >= N_GENES, uniq - N_GENES, uniq)   # id-table row
    ue = gmap[np.clip(urow, 0, N_GENES - 1)]
    uvalid = (uniq < N_GENES) & (ue > 0) & (ue < V_ESM)
    ueidx = np.where(uvalid, ue, V_ESM)                      # row V_ESM is zero pad

    eidx_core = np.full((n_cores, ng_cap), V_ESM, np.int16)
    idid_core = np.zeros((n_cores, ng_cap), np.int16)
    mask_core = np.zeros((n_cores, ng_cap), BF16)
    eidx_core[core_of, slot_of] = ueidx.astype(np.int16)
    idid_core[core_of, slot_of] = urow.astype(np.int16)
    mask_core[core_of, slot_of] = uvalid.astype(BF16)

    # tokens -> SEL one-hots: column j of (core, group) = j-th token of that
    # bin in flat order.  SEL stored partition-major: sel[p, q*TPG+j].
    tok_bin = bin_of[inv]
    tok_rank = rank_of[inv]
    bin_sort = np.argsort(tok_bin, kind="stable")  # flat positions by bin
    bcnt = np.bincount(tok_bin, minlength=NB)
    boff = np.concatenate([[0], np.cumsum(bcnt)])
    sel_core = np.zeros((n_cores, 128, n_groups * TPG), BF16)
    tok_pos = [[None] * n_groups for _ in range(n_cores)]
    for b in range(NB):
        pos = bin_sort[boff[b]:boff[b + 1]]
        cc, q = b % n_cores, b // n_cores
        tok_pos[cc][q] = pos
        sel_core[cc, tok_rank[pos], q * TPG + np.arange(len(pos))] = 1

    # host weight folding
    tg = np.tanh(float(np.asarray(gate).reshape(-1)[0]))
    Wp64 = np.asarray(Wp, np.float64)
    W1b = np.asarray(W1, np.float64)[ID_DIM:, :]
    Wc = tg * (Wp64 @ W1b)                                   # [1280, 256]
    cb = tg * (np.asarray(bp, np.float64) @ W1b)             # [256]
    has_cb = bool(np.abs(cb).max() > 1e-12)

    hot = np.empty((128, 10 * PROJ + PROJ + 2 * ID_DIM), BF16)
    hot[:, 0:10 * PROJ] = Wc.reshape(10, 128, PROJ).transpose(1, 0, 2) \
                            .reshape(128, 10 * PROJ).astype(BF16)
    hot[:, 10 * PROJ:11 * PROJ] = np.asarray(W1[:ID_DIM, :]).astype(BF16)
    hot[:, 11 * PROJ:] = np.asarray(W2).reshape(2, 128, ID_DIM) \
                           .transpose(1, 0, 2).reshape(128, 2 * ID_DIM).astype(BF16)
    f32w = np.empty((128, 130), np.float32)
    f32w[:, 0:2] = np.asarray(b1).astype(np.float32).reshape(2, 128).T
    f32w[:, 2:] = np.tile(np.asarray(b2).astype(np.float32).reshape(1, 128), (128, 1))

    shared = {
        "esmbf": np.concatenate(
            [np.asarray(esm_table).astype(BF16), np.zeros((1, ESM_DIM), BF16)], axis=0),
        "idbf": np.asarray(id_table).astype(BF16),
        "hotbf": hot,
        "f32w": f32w,
    }
    per_core = []
    for cc in range(n_cores):
        pc = {
            "idx16": np.concatenate(
                [_wrap16(eidx_core[cc]), _wrap16(idid_core[cc])], axis=1),
            "selbf": sel_core[cc],
        }
        if has_cb:
            pc["mcbbf"] = np.concatenate(
                [mask_core[cc], cb.astype(BF16)]).reshape(1, -1).copy()
        per_core.append(pc)
    return shared, per_core, tok_pos, n_groups, has_cb


def kernel(idx, gene_idx_to_esm_idx, id_table, esm_table, Wp, bp, gate,
           W1, b1, W2, b2, _trace=False, **_run_kwargs):
    from concourse.bass_utils import run_bass_kernel_spmd

    shared, per_core, tok_pos, n_groups, has_cb = prepare_host(
        idx, gene_idx_to_esm_idx, id_table, esm_table, Wp, bp, gate, W1, b1, W2, b2)
    bkey = (n_groups, has_cb)
    if bkey not in _BUILD_CACHE:
        _BUILD_CACHE[bkey] = _build_best(n_groups, has_cb)
    nc = _BUILD_CACHE[bkey]

    in_maps = [dict(shared, **pc) for pc in per_core]
    res = run_bass_kernel_spmd(nc, in_maps, list(range(N_CORES)), trace=_trace,
                               **_run_kwargs)
    sh = np.asarray(idx).shape
    out = np.empty((NTOK_TOTAL, ID_DIM), np.float32)
    for c in range(N_CORES):
        arr = np.asarray(res.results[c]["out"]).astype(np.float32)  # [128, G*TPG]
        for q in range(n_groups):
            pos = tok_pos[c][q]
            if len(pos):
                out[pos] = arr[:, q * TPG:q * TPG + len(pos)].T
    out = out.reshape(sh[0], sh[1], ID_DIM)
    if _trace:
        return out, res
    return out
